# revision 1
# baseline (speedup 1.0000x reference)
"""Trainium2 Bass kernel for nn_MixtureOfExperts_29867202576447.

Strategy: data-parallel over tokens (8 cores x 512 tokens), dense expert
compute with top-2 gates applied as a mask, float32r (tf32) matmuls,
activations kept in [channels, tokens] layout so every GEMM/conv consumes
the previous layer's output directly (weights stationary as lhsT).

Self-contained: hardcodes all shapes; host-side prep only shards/pads x,
transposes the small conv weights, and packs biases.
"""
import numpy as np
from contextlib import ExitStack

import concourse.bass as bass
import concourse.tile as tile
import concourse.mybir as mybir
from concourse.bass_utils import run_bass_kernel_spmd

F32R = mybir.dt.float32r
F32 = mybir.dt.float32
AF = mybir.ActivationFunctionType
OP = mybir.AluOpType
AX = mybir.AxisListType

D_IN, D_HID, E = 512, 2048, 10
B, T = 2, 2048
TPC = 512          # tokens per core
HALO = 64          # halo columns on each side of the token window
W = TPC + 2 * HALO  # 640 buffer columns per core
NCORES = 8

_ctr = [0]


def _split_multi_waits(nc, max_waits=1):
    """walrus here accepts one sync-wait per instruction; hoist extras onto
    same-engine NoOps placed immediately before the instruction."""
    n = 0
    for f in nc.m.functions:
        for bb in f.blocks:
            out = []
            changed = False
            for ins in bb.instructions:
                si = getattr(ins, "sync_info", None)
                waits = list(si.on_wait) if (si is not None and si.on_wait) else []
                if len(waits) > max_waits:
                    for w in waits[:-max_waits]:
                        _ctr[0] += 1
                        nop = mybir.InstNoOp(
                            name=f"I-waitsplit-{_ctr[0]}", engine=ins.engine,
                            ins=[], outs=[])
                        nop.sync_info = mybir.SyncInfo(on_wait=[w], on_update=[])
                        nc.register_instruction(nop)
                        out.append(nop)
                    si.on_wait = waits[-max_waits:]
                    changed = True
                    n += 1
                out.append(ins)
            if changed:
                bb.instructions = out
    return n


def _build(reps=1):
    nc = bass.Bass(trn_type="TRN2")

    # ---------------- DRAM I/O ----------------
    xt = nc.dram_tensor("xt", [D_IN, W], F32R, kind="ExternalInput")
    rcw1t = nc.dram_tensor("rcw1t", [3, D_IN, D_IN], F32R, kind="ExternalInput")
    rcw2t = nc.dram_tensor("rcw2t", [3, D_IN, E], F32R, kind="ExternalInput")
    shw1t = nc.dram_tensor("shw1t", [9, D_IN, D_HID], F32R, kind="ExternalInput")
    shw2t = nc.dram_tensor("shw2t", [D_HID, D_IN], F32R, kind="ExternalInput")
    ew1 = nc.dram_tensor("ew1", [E, D_IN, D_HID], F32R, kind="ExternalInput")
    ew2 = nc.dram_tensor("ew2", [E, D_HID, D_IN], F32R, kind="ExternalInput")
    rcb1 = nc.dram_tensor("rcb1", [128, 4], F32, kind="ExternalInput")
    rcb2 = nc.dram_tensor("rcb2", [E, 1], F32, kind="ExternalInput")
    shb1 = nc.dram_tensor("shb1", [128, 16], F32, kind="ExternalInput")
    shb2 = nc.dram_tensor("shb2", [128, 4], F32, kind="ExternalInput")
    eb1p = nc.dram_tensor("eb1p", [E, 128, 16], F32, kind="ExternalInput")
    ebcp = nc.dram_tensor("ebcp", [E, 128, 4], F32, kind="ExternalInput")
    lng = nc.dram_tensor("lng", [1, D_IN], F32R, kind="ExternalInput")
    lnb = nc.dram_tensor("lnb", [1, D_IN], F32R, kind="ExternalInput")
    ones = nc.dram_tensor("ones", [1, 128], F32R, kind="ExternalInput")
    sel = nc.dram_tensor("sel", [E, E * 128], F32R, kind="ExternalInput")
    ident = nc.dram_tensor("ident", [128, 128], F32, kind="ExternalInput")
    rcmask = nc.dram_tensor("rcmask", [1, 528], F32R, kind="ExternalInput")
    yout = nc.dram_tensor("yout", [TPC, D_IN], F32, kind="ExternalOutput")

    C0 = HALO            # buffer col of first center token
    with tile.TileContext(nc) as tc:
      for rep in range(reps):
       with ExitStack() as ctx:
        R = f"r{rep}_"
        const = ctx.enter_context(tc.tile_pool(name=R + "const", bufs=1))
        acts = ctx.enter_context(tc.tile_pool(name=R + "acts", bufs=1))
        wstream = ctx.enter_context(tc.tile_pool(name=R + "wstream", bufs=10))
        scratch = ctx.enter_context(tc.tile_pool(name=R + "scratch", bufs=2))
        hpool = ctx.enter_context(tc.tile_pool(name=R + "hpool", bufs=18))
        psum = ctx.enter_context(tc.tile_pool(name=R + "psum", bufs=7, space="PSUM"))
        pst = psum

        # ---------------- constants / x ----------------
        id_sb = const.tile([128, 128], F32)
        nc.sync.dma_start(id_sb[:], ident[:])
        ones_sb = const.tile([1, 128], F32R)
        nc.sync.dma_start(ones_sb[:], ones[:])
        sel_sb = const.tile([E, E * 128], F32R)
        nc.sync.dma_start(sel_sb[:], sel[:])
        rcb1_sb = const.tile([128, 4], F32)
        nc.sync.dma_start(rcb1_sb[:], rcb1[:])
        rcb2_sb = const.tile([E, 1], F32)
        nc.sync.dma_start(rcb2_sb[:], rcb2[:])
        shb1_sb = const.tile([128, 16], F32)
        nc.sync.dma_start(shb1_sb[:], shb1[:])
        shb2_sb = const.tile([128, 4], F32)
        nc.sync.dma_start(shb2_sb[:], shb2[:])
        lng_r = const.tile([1, D_IN], F32R)
        nc.sync.dma_start(lng_r[:], lng[:])
        lnb_r = const.tile([1, D_IN], F32R)
        nc.sync.dma_start(lnb_r[:], lnb[:])

        xt_sb = []
        for k in range(4):
            t = acts.tile([128, W], F32R, tag=f"xt{k}")
            nc.sync.dma_start(t[:], xt[k * 128:(k + 1) * 128, :])
            xt_sb.append(t)

        # ln gamma/beta broadcast to 128 partitions
        lng_bc = const.tile([128, D_IN], F32)
        lnb_bc = const.tile([128, D_IN], F32)
        for src, dst in ((lng_r, lng_bc), (lnb_r, lnb_bc)):
            p = pst.tile([128, D_IN], F32, tag="aux", bufs=1, name="p_lnbc")
            nc.tensor.matmul(p[:], ones_sb[:], src[:], start=True, stop=True)
            nc.scalar.copy(dst[:], p[:])

        # rc edge mask: zero rc1 at columns outside the sequence so the
        # second routing conv sees the same zero padding as the reference
        rcm_r = const.tile([1, 528], F32R)
        nc.sync.dma_start(rcm_r[:], rcmask[:])
        rcm_bc = const.tile([128, 528], F32R)
        for t0 in (0, 264):
            pmask = psum.tile([128, 264], F32, tag="aux", bufs=1, name="pmask")
            nc.tensor.matmul(pmask[:], ones_sb[:], rcm_r[:, t0:t0 + 264],
                             start=True, stop=True)
            nc.vector.tensor_scalar(rcm_bc[:, t0:t0 + 264], pmask[:], 0.0, None,
                                    op0=OP.add)

        # =========== shared conv1: sh1 = silu(conv9(x)), [16][128, 512] ===========
        sh1_sb = []
        for m in range(16):
            t = acts.tile([128, TPC], F32R, tag=f"sh1_{m}")
            sh1_sb.append(t)
        for mb in range(4):          # m-blocks of 4 chunks
            plist = [psum.tile([128, TPC], F32, tag="mm", name=f"psh_{mb}_{i}") for i in range(4)]
            first = True
            for tap in range(9):
                for k in range(4):
                    wsub = wstream.tile([128, 512], F32R, tag="w", name="wsub")
                    nc.sync.dma_start(
                        wsub[:], shw1t[tap, k * 128:(k + 1) * 128,
                                       mb * 512:(mb + 1) * 512])
                    for mi in range(4):
                        nc.tensor.matmul(
                            plist[mi][:], wsub[:, mi * 128:(mi + 1) * 128],
                            xt_sb[k][:, C0 + tap - 4: C0 + tap - 4 + TPC],
                            start=first, stop=(tap == 8 and k == 3))
                    first = False
            for mi in range(4):
                m = mb * 4 + mi
                nc.scalar.activation(sh1_sb[m][:], plist[mi][:], AF.Silu,
                                     bias=shb1_sb[:, m:m + 1])

        # =========== shared conv2 (k=1): sh2 psum kept for final combine ===========
        sh2_sb = []
        for mo in range(4):
            t = acts.tile([128, TPC], F32, tag=f"sh2_{mo}")
            sh2_sb.append(t)
        s2list = [psum.tile([128, TPC], F32, tag="mm", name=f"ps2_{i}") for i in range(4)]
        for k in range(16):
            wsub = wstream.tile([128, 512], F32R, tag="w", name="wsub")
            nc.sync.dma_start(wsub[:], shw2t[k * 128:(k + 1) * 128, :])
            for mo in range(4):
                nc.tensor.matmul(s2list[mo][:], wsub[:, mo * 128:(mo + 1) * 128],
                                 sh1_sb[k][:], start=(k == 0), stop=(k == 15))
        for mo in range(4):
            nc.scalar.activation(sh2_sb[mo][:], s2list[mo][:], AF.Identity,
                                 bias=shb2_sb[:, mo:mo + 1])

        # =========== routing conv1: rc1 = gelu(conv3(x, rc_w1)) ===========
        # computed over buffer cols [56, 584) -> rc1 col c == buffer col 56+c
        RC_LO, RC_W = 56, 528
        with tc.tile_pool(name=R + "routing", bufs=1) as rpool:
            rcw2_sb = {}
            for tap in range(3):
                for k in range(4):
                    t = rpool.tile([128, E], F32R, tag=f"rcw2_{tap}_{k}")
                    nc.sync.dma_start(t[:], rcw2t[tap, k * 128:(k + 1) * 128, :])
                    rcw2_sb[tap, k] = t

            rc1_sb = []
            for m in range(4):
                t = rpool.tile([128, RC_W], F32R, tag=f"rc1_{m}")
                rc1_sb.append(t)
            for t0, tw in ((0, 264), (264, 264)):
                plist = [psum.tile([128, tw], F32, tag="mm", name=f"prc_{t0}_{i}")
                         for i in range(4)]
                for tap in range(3):
                    for k in range(4):
                        wsub = wstream.tile([128, 512], F32R, tag="w", name="wsub")
                        nc.sync.dma_start(wsub[:], rcw1t[tap, k * 128:(k + 1) * 128, :])
                        for m in range(4):
                            nc.tensor.matmul(
                                plist[m][:], wsub[:, m * 128:(m + 1) * 128],
                                xt_sb[k][:, RC_LO + t0 + tap - 1: RC_LO + t0 + tap - 1 + tw],
                                start=(tap == 0 and k == 0), stop=(tap == 2 and k == 3))
                for m in range(4):
                    nc.scalar.activation(rc1_sb[m][:, t0:t0 + tw], plist[m][:], AF.Gelu,
                                         bias=rcb1_sb[:, m:m + 1])
                    nc.vector.tensor_tensor(rc1_sb[m][:, t0:t0 + tw],
                                            rc1_sb[m][:, t0:t0 + tw],
                                            rcm_bc[:, t0:t0 + tw], op=OP.mult)

            # ======= routing conv2 -> logits [E, 512] (center tokens) =======
            lgp = pst.tile([E, TPC], F32, tag="aux", bufs=1, name="lgp")
            first = True
            for tap in range(3):
                for k in range(4):
                    nc.tensor.matmul(
                        lgp[:], rcw2_sb[tap, k][:],
                        rc1_sb[k][:, (C0 - RC_LO) + tap - 1: (C0 - RC_LO) + tap - 1 + TPC],
                        start=first, stop=(tap == 2 and k == 3))
                    first = False
            lg_sb = rpool.tile([E, TPC], F32, tag="lg")
            nc.scalar.activation(lg_sb[:], lgp[:], AF.Identity, bias=rcb2_sb[:])

            # ======= top-2 gating -> gatesT [E, 512] (f32r) =======
            gatesT = acts.tile([E, TPC], F32R)
            for tt in range(4):
                tp = pst.tile([128, E], F32, tag="aux", bufs=1, name="tp")
                nc.tensor.transpose(tp[:], lg_sb[:, tt * 128:(tt + 1) * 128],
                                    id_sb[0:E, 0:E])
                lT = scratch.tile([128, E], F32, tag="lT")
                nc.scalar.copy(lT[:], tp[:])
                m1 = scratch.tile([128, 1], F32, tag="m1")
                nc.vector.reduce_max(m1[:], lT[:], axis=AX.X)
                mask1 = scratch.tile([128, E], F32, tag="mask1")
                nc.vector.tensor_scalar(mask1[:], lT[:], m1[:], None, op0=OP.is_equal)
                lmask = scratch.tile([128, E], F32, tag="lmask")
                nc.vector.scalar_tensor_tensor(lmask[:], mask1[:], -1e30, lT[:],
                                               op0=OP.mult, op1=OP.add)
                m2 = scratch.tile([128, 1], F32, tag="m2")
                nc.vector.reduce_max(m2[:], lmask[:], axis=AX.X)
                mask2 = scratch.tile([128, E], F32, tag="mask2")
                nc.vector.tensor_scalar(mask2[:], lmask[:], m2[:], None, op0=OP.is_equal)
                d = scratch.tile([128, 1], F32, tag="d")
                nc.vector.tensor_scalar(d[:], m2[:], m1[:], None, op0=OP.subtract)
                e_ = scratch.tile([128, 1], F32, tag="e_")
                nc.scalar.activation(e_[:], d[:], AF.Exp)
                ope = scratch.tile([128, 1], F32, tag="ope")
                nc.vector.tensor_scalar(ope[:], e_[:], 1.0, None, op0=OP.add)
                g1 = scratch.tile([128, 1], F32, tag="g1")
                nc.vector.reciprocal(g1[:], ope[:])
                g2 = scratch.tile([128, 1], F32, tag="g2")
                nc.vector.tensor_scalar(g2[:], g1[:], -1.0, 1.0, op0=OP.mult, op1=OP.add)
                t1 = scratch.tile([128, E], F32, tag="t1")
                nc.vector.tensor_scalar(t1[:], mask1[:], g1[:], None, op0=OP.mult)
                gt = scratch.tile([128, E], F32, tag="gt")
                nc.vector.scalar_tensor_tensor(gt[:], mask2[:], g2[:], t1[:],
                                               op0=OP.mult, op1=OP.add)
                gp = pst.tile([E, 128], F32, tag="aux", bufs=1, name="gp")
                nc.tensor.transpose(gp[:], gt[:], id_sb[:])
                nc.vector.tensor_scalar(gatesT[:, tt * 128:(tt + 1) * 128], gp[:],
                                        0.0, None, op0=OP.add)

        # =========== experts (dense, gated accumulate into y_acc) ===========
        y_acc = [acts.tile([128, TPC], F32, tag=f"y{mo}", name=f"y_acc_{mo}") for mo in range(4)]
        for e in range(E):
            b1 = scratch.tile([128, 16], F32, tag="b1")
            nc.sync.dma_start(b1[:], eb1p[e])
            bce = scratch.tile([128, 4], F32, tag="bce")
            nc.sync.dma_start(bce[:], ebcp[e])

            h_sb = []
            for mb in range(4):
                plist = [psum.tile([128, TPC], F32, tag="mm", name=f"ph_{e}_{mb}_{i}") for i in range(4)]
                for k in range(4):
                    wsub = wstream.tile([128, 512], F32R, tag="w", name="wsub")
                    nc.sync.dma_start(
                        wsub[:], ew1[e, k * 128:(k + 1) * 128,
                                     mb * 512:(mb + 1) * 512])
                    for mi in range(4):
                        nc.tensor.matmul(
                            plist[mi][:], wsub[:, mi * 128:(mi + 1) * 128],
                            xt_sb[k][:, C0:C0 + TPC],
                            start=(k == 0), stop=(k == 3))
                for mi in range(4):
                    m = mb * 4 + mi
                    # elu(v) = max(v, min(exp(v), 1) - 1), v = h + b1
                    eh = scratch.tile([128, TPC], F32, tag="he")
                    nc.scalar.activation(eh[:], plist[mi][:], AF.Exp,
                                         bias=b1[:, m:m + 1])
                    em = scratch.tile([128, TPC], F32, tag="hm")
                    nc.vector.tensor_scalar(em[:], eh[:], 1.0, -1.0,
                                            op0=OP.min, op1=OP.add)
                    h1 = hpool.tile([128, TPC], F32R, tag="h")
                    nc.vector.scalar_tensor_tensor(h1[:], plist[mi][:],
                                                   b1[:, m:m + 1], em[:],
                                                   op0=OP.add, op1=OP.max)
                    h_sb.append(h1)

            # broadcast this expert's gate row
            bcp = pst.tile([128, TPC], F32, tag="aux", bufs=1, name="bcp")
            nc.tensor.matmul(bcp[:], sel_sb[:, e * 128:(e + 1) * 128], gatesT[:],
                             start=True, stop=True)
            bc_sb = scratch.tile([128, TPC], F32, tag="bc")
            nc.scalar.copy(bc_sb[:], bcp[:])

            elist = [psum.tile([128, TPC], F32, tag="mm", name=f"pe_{e}_{i}") for i in range(4)]
            for k in range(16):
                wsub = wstream.tile([128, 512], F32R, tag="w", name="wsub")
                nc.sync.dma_start(wsub[:], ew2[e, k * 128:(k + 1) * 128, :])
                for mo in range(4):
                    nc.tensor.matmul(elist[mo][:], wsub[:, mo * 128:(mo + 1) * 128],
                                     h_sb[k][:], start=(k == 0), stop=(k == 15))
            for mo in range(4):
                # (eo + (e_b2 - colsum)) * gate, accumulated into y_acc
                if e == 0:
                    nc.vector.scalar_tensor_tensor(
                        y_acc[mo][:], elist[mo][:], bce[:, mo:mo + 1], bc_sb[:],
                        op0=OP.add, op1=OP.mult)
                else:
                    yt = scratch.tile([128, TPC], F32, tag="yt")
                    nc.vector.scalar_tensor_tensor(
                        yt[:], elist[mo][:], bce[:, mo:mo + 1], bc_sb[:],
                        op0=OP.add, op1=OP.mult)
                    nc.vector.tensor_tensor(y_acc[mo][:], y_acc[mo][:], yt[:],
                                            op=OP.add)

        # =========== z = x + y + sh2 ; transpose ; layernorm ; out ===========
        z_sb = []
        for mo in range(4):
            z = acts.tile([128, TPC], F32, tag=f"z{mo}")
            nc.vector.tensor_tensor(z[:], y_acc[mo][:], sh2_sb[mo][:], op=OP.add)
            nc.vector.tensor_tensor(z[:], z[:],
                                    xt_sb[mo][:, C0:C0 + TPC].bitcast(F32), op=OP.add)
            z_sb.append(z)

        for tt in range(4):
            zT = scratch.tile([128, D_IN], F32, tag="zT")
            for mo in range(4):
                ztp = pst.tile([128, 128], F32, tag="aux", bufs=1, name="ztp")
                nc.tensor.transpose(ztp[:], z_sb[mo][:, tt * 128:(tt + 1) * 128],
                                    id_sb[:])
                nc.scalar.copy(zT[:, mo * 128:(mo + 1) * 128], ztp[:])
            srow = scratch.tile([128, 1], F32, tag="srow")
            nc.vector.reduce_sum(srow[:], zT[:], axis=AX.X)
            nmean = scratch.tile([128, 1], F32, tag="nmean")
            nc.vector.tensor_scalar(nmean[:], srow[:], -1.0 / D_IN, None, op0=OP.mult)
            zc = scratch.tile([128, D_IN], F32, tag="zc")
            nc.vector.tensor_scalar(zc[:], zT[:], nmean[:], None, op0=OP.add)
            sq = scratch.tile([128, D_IN], F32, tag="sq")
            ssq = scratch.tile([128, 1], F32, tag="ssq")
            nc.scalar.activation(sq[:], zc[:], AF.Square, accum_out=ssq[:])
            vpe = scratch.tile([128, 1], F32, tag="vpe")
            nc.vector.tensor_scalar(vpe[:], ssq[:], 1.0 / D_IN, 1e-5,
                                    op0=OP.mult, op1=OP.add)
            rinv = scratch.tile([128, 1], F32, tag="rinv")
            nc.vector.reciprocal(rinv[:], vpe[:])
            rstd = scratch.tile([128, 1], F32, tag="rstd")
            nc.scalar.activation(rstd[:], rinv[:], AF.Sqrt)
            normed = scratch.tile([128, D_IN], F32, tag="normed")
            nc.vector.tensor_scalar(normed[:], zc[:], rstd[:], None, op0=OP.mult)
            og = scratch.tile([128, D_IN], F32, tag="og")
            nc.vector.tensor_tensor(og[:], normed[:], lng_bc[:], op=OP.mult)
            out = scratch.tile([128, D_IN], F32, tag="out")
            nc.vector.tensor_tensor(out[:], og[:], lnb_bc[:], op=OP.add)
            nc.sync.dma_start(yout[tt * 128:(tt + 1) * 128, :], out[:])

    _split_multi_waits(nc)
    return nc


_CACHE = {}


def _get_nc(reps=1):
    key = f"nc{reps}"
    if key not in _CACHE:
        _CACHE[key] = _build(reps)
    return _CACHE[key]


LAST_RESULT = {}


def kernel(x, rc_w1, rc_b1, rc_w2, rc_b2, sh_w1, sh_b1, sh_w2, sh_b2,
           e_w1, e_b1, e_w2, e_b2, ln_g, ln_b, **kwargs):
    x = np.asarray(x, np.float32)
    f = lambda a: np.ascontiguousarray(np.asarray(a, np.float32))

    shared = {
        "rcw1t": f(np.asarray(rc_w1, np.float32).transpose(2, 1, 0)),
        "rcw2t": f(np.asarray(rc_w2, np.float32).transpose(2, 1, 0)),
        "shw1t": f(np.asarray(sh_w1, np.float32).transpose(2, 1, 0)),
        "shw2t": f(np.asarray(sh_w2, np.float32)[:, :, 0].T),
        "ew1": f(e_w1),
        "ew2": f(e_w2),
        "rcb1": f(np.asarray(rc_b1, np.float32).reshape(4, 128).T),
        "rcb2": f(np.asarray(rc_b2, np.float32).reshape(E, 1)),
        "shb1": f(np.asarray(sh_b1, np.float32).reshape(16, 128).T),
        "shb2": f(np.asarray(sh_b2, np.float32).reshape(4, 128).T),
        "eb1p": f(np.asarray(e_b1, np.float32).reshape(E, 16, 128).transpose(0, 2, 1)),
        "ebcp": f(np.asarray(e_b2, np.float32).reshape(E, 4, 128).transpose(0, 2, 1)),
        "lng": f(np.asarray(ln_g, np.float32).reshape(1, D_IN)),
        "lnb": f(np.asarray(ln_b, np.float32).reshape(1, D_IN)),
        "ones": np.ones((1, 128), np.float32),
        "sel": np.repeat(np.eye(E, dtype=np.float32), 128, axis=1),
        "ident": np.eye(128, dtype=np.float32),
    }

    in_maps = []
    for c in range(NCORES):
        b, j = divmod(c, T // TPC)
        lo_tok = j * TPC - HALO
        hi_tok = j * TPC + TPC + HALO
        xh = np.zeros((W, D_IN), np.float32)
        lo = max(0, lo_tok)
        hi = min(T, hi_tok)
        xh[lo - lo_tok: hi - lo_tok] = x[b, lo:hi]
        im = dict(shared)
        im["xt"] = np.ascontiguousarray(xh.T)
        rcm = np.zeros((1, 528), np.float32)
        for cidx in range(528):
            tok = j * TPC - 8 + cidx
            if 0 <= tok < T:
                rcm[0, cidx] = 1.0
        im["rcmask"] = rcm
        in_maps.append(im)

    nc = _get_nc()
    res = run_bass_kernel_spmd(nc, in_maps, core_ids=list(range(NCORES)),
                               **kwargs)
    LAST_RESULT["res"] = res

    out = np.empty((B, T, D_IN), np.float32)
    for c in range(NCORES):
        b, j = divmod(c, T // TPC)
        out[b, j * TPC:(j + 1) * TPC] = res.results[c]["yout"]
    return out



# revision 12
# speedup vs baseline: 1.2472x; 1.2472x over previous
"""Trainium2 Bass kernel for nn_MixtureOfExperts_29867202576447.

Strategy: data-parallel over tokens (8 cores x 512 tokens).
 - Routing convs + top-2 gating: float32r (exact top-k selection).
 - Shared conv1 (9-tap, 512->2048): fp8 DoubleRow matmuls with 3-term
   error compensation (e4m3 hi values + e5m2 residuals for both x and W),
   all three terms accumulated in one PSUM bank. SiLU -> bf16.
 - Shared conv2 (1x1, 2048->512): bf16.
 - Experts (10x dense, gated): plain fp8e4m3 DoubleRow for both GEMMs.
 - Final residual + layernorm in f32.

Self-contained: hardcodes all shapes; host-side prep shards/pads x,
quantizes + packs weights for the DoubleRow K-pair layout.
"""
import numpy as np
from contextlib import ExitStack

import concourse.bass as bass
import concourse.tile as tile
import concourse.mybir as mybir
from concourse.bass_utils import run_bass_kernel_spmd

F32R = mybir.dt.float32r
F32 = mybir.dt.float32
BF16 = mybir.dt.bfloat16
F8 = mybir.dt.float8e4
F8L = mybir.dt.float8e5
AF = mybir.ActivationFunctionType
OP = mybir.AluOpType
AX = mybir.AxisListType
DR = mybir.MatmulPerfMode.DoubleRow

D_IN, D_HID, E = 512, 2048, 10
B, T = 2, 2048
TPC = 512          # tokens per core
HALO = 64          # halo columns each side of the f32r token window
W = TPC + 2 * HALO  # 640 buffer columns (routing/residual slab)
H8 = 4             # halo of the fp8 slab (9-tap conv)
W8 = TPC + 2 * H8  # 520 columns
NCORES = 8

_ctr = [0]


def _split_multi_waits(nc, max_waits=1):
    """walrus accepts one sync-wait per instruction; hoist extras onto
    same-engine NoOps placed immediately before the instruction."""
    n = 0
    for f in nc.m.functions:
        for bb in f.blocks:
            out = []
            changed = False
            for ins in bb.instructions:
                si = getattr(ins, "sync_info", None)
                waits = list(si.on_wait) if (si is not None and si.on_wait) else []
                if len(waits) > max_waits:
                    for w in waits[:-max_waits]:
                        _ctr[0] += 1
                        nop = mybir.InstNoOp(
                            name=f"I-waitsplit-{_ctr[0]}", engine=ins.engine,
                            ins=[], outs=[])
                        nop.sync_info = mybir.SyncInfo(on_wait=[w], on_update=[])
                        nc.register_instruction(nop)
                        out.append(nop)
                    si.on_wait = waits[-max_waits:]
                    changed = True
                    n += 1
                out.append(ins)
            if changed:
                bb.instructions = out
    return n


DEBUG = [False]


def _build(reps=1):
    nc = bass.Bass(trn_type="TRN2")

    # ---------------- DRAM I/O ----------------
    xt = nc.dram_tensor("xt", [D_IN, W], F32R, kind="ExternalInput")
    x8h = nc.dram_tensor("x8h", [128, 4, W8], F8, kind="ExternalInput")
    x8l = nc.dram_tensor("x8l", [128, 4, W8], F8L, kind="ExternalInput")
    rcw1t = nc.dram_tensor("rcw1t", [3, D_IN, D_IN], F32R, kind="ExternalInput")
    rcw2t = nc.dram_tensor("rcw2t", [3, D_IN, E], F32R, kind="ExternalInput")
    # shared conv1 weights: [tap, kp, blk, p, j, mcol] (m in 2 blocks of 1024)
    shw1h = nc.dram_tensor("shw1h", [9, 2, 2, 128, 2, 1024], F8, kind="ExternalInput")
    shw1l = nc.dram_tensor("shw1l", [9, 2, 2, 128, 2, 1024], F8L, kind="ExternalInput")
    # shared conv2 weights bf16: [g, p, kk, d]
    shw2p = nc.dram_tensor("shw2p", [4, 128, 4, D_IN], BF16, kind="ExternalInput")
    # expert weights fp8, DoubleRow K-pair layout
    ew1p = nc.dram_tensor("ew1p", [E, 128, 2, 2, D_HID], F8, kind="ExternalInput")
    ew2p = nc.dram_tensor("ew2p", [E, 128, 8, 2, D_IN], F8, kind="ExternalInput")
    rcb1 = nc.dram_tensor("rcb1", [128, 4], F32, kind="ExternalInput")
    rcb2 = nc.dram_tensor("rcb2", [E, 1], F32, kind="ExternalInput")
    shb1 = nc.dram_tensor("shb1", [128, 16], F32, kind="ExternalInput")
    shb2 = nc.dram_tensor("shb2", [128, 4], F32, kind="ExternalInput")
    eb1p = nc.dram_tensor("eb1p", [E, 128, 16], F32, kind="ExternalInput")
    ebcp = nc.dram_tensor("ebcp", [E, 128, 4], F32, kind="ExternalInput")
    lng = nc.dram_tensor("lng", [1, D_IN], F32R, kind="ExternalInput")
    lnb = nc.dram_tensor("lnb", [1, D_IN], F32R, kind="ExternalInput")
    ones = nc.dram_tensor("ones", [1, 128], F32R, kind="ExternalInput")
    sel = nc.dram_tensor("sel", [E, E * 128], F32R, kind="ExternalInput")
    ident = nc.dram_tensor("ident", [128, 128], F32, kind="ExternalInput")
    rcmask = nc.dram_tensor("rcmask", [1, 528], F32R, kind="ExternalInput")
    yout = nc.dram_tensor("yout", [TPC, D_IN], F32, kind="ExternalOutput")
    if DEBUG[0]:
        dbg_lg = nc.dram_tensor("dbg_lg", [E, TPC], F32, kind="ExternalOutput")
        dbg_g = nc.dram_tensor("dbg_g", [E, TPC], F32, kind="ExternalOutput")
        dbg_sh1 = nc.dram_tensor("dbg_sh1", [128, 16, TPC], BF16, kind="ExternalOutput")
        dbg_sh2 = nc.dram_tensor("dbg_sh2", [128, TPC], F32, kind="ExternalOutput")
        dbg_y = nc.dram_tensor("dbg_y", [128, TPC], F32, kind="ExternalOutput")
        dbg_h8 = nc.dram_tensor("dbg_h8", [128, 8, 2, TPC], F8, kind="ExternalOutput")

    C0 = HALO            # f32r-slab col of first center token
    with tile.TileContext(nc) as tc:
      for rep in range(reps):
       with ExitStack() as ctx:
        R = f"r{rep}_"
        const = ctx.enter_context(tc.tile_pool(name=R + "const", bufs=1))
        acts = ctx.enter_context(tc.tile_pool(name=R + "acts", bufs=1))
        wstream = ctx.enter_context(tc.tile_pool(name=R + "wstream", bufs=3))
        scratch = ctx.enter_context(tc.tile_pool(name=R + "scratch", bufs=2))
        epool = ctx.enter_context(tc.tile_pool(name=R + "epool", bufs=2))
        psum = ctx.enter_context(tc.tile_pool(name=R + "psum", bufs=8, space="PSUM"))

        # ---------------- constants / x ----------------
        id_sb = const.tile([128, 128], F32)
        nc.sync.dma_start(id_sb[:], ident[:])
        ones_sb = const.tile([1, 128], F32R)
        nc.sync.dma_start(ones_sb[:], ones[:])
        sel_sb = const.tile([E, E * 128], F32R)
        nc.sync.dma_start(sel_sb[:], sel[:])
        rcb1_sb = const.tile([128, 4], F32)
        nc.sync.dma_start(rcb1_sb[:], rcb1[:])
        rcb2_sb = const.tile([E, 1], F32)
        nc.sync.dma_start(rcb2_sb[:], rcb2[:])
        shb1_sb = const.tile([128, 16], F32)
        nc.sync.dma_start(shb1_sb[:], shb1[:])
        shb2_sb = const.tile([128, 4], F32)
        nc.sync.dma_start(shb2_sb[:], shb2[:])
        lng_r = const.tile([1, D_IN], F32R)
        nc.sync.dma_start(lng_r[:], lng[:])
        lnb_r = const.tile([1, D_IN], F32R)
        nc.sync.dma_start(lnb_r[:], lnb[:])

        xt_sb = []
        for k in range(4):
            t = acts.tile([128, W], F32R, tag=f"xt{k}", name=f"xt_sb{k}")
            nc.sync.dma_start(t[:], xt[k * 128:(k + 1) * 128, :])
            xt_sb.append(t)
        x8h_sb = acts.tile([128, 4, W8], F8, tag="x8h")
        nc.sync.dma_start(x8h_sb[:], x8h[:])
        x8l_sb = acts.tile([128, 4, W8], F8L, tag="x8l")
        nc.sync.dma_start(x8l_sb[:], x8l[:])

        # ln gamma/beta broadcast to 128 partitions
        lng_bc = const.tile([128, D_IN], F32)
        lnb_bc = const.tile([128, D_IN], F32)
        for src, dst in ((lng_r, lng_bc), (lnb_r, lnb_bc)):
            p = psum.tile([128, D_IN], F32, tag="mm", name="p_lnbc")
            nc.tensor.matmul(p[:], ones_sb[:], src[:], start=True, stop=True)
            nc.scalar.copy(dst[:], p[:])

        # rc edge mask: zero rc1 at columns outside the sequence
        rcm_r = const.tile([1, 528], F32R)
        nc.sync.dma_start(rcm_r[:], rcmask[:])
        rcm_bc = const.tile([128, 528], F32R)
        for t0 in (0, 264):
            pmask = psum.tile([128, 264], F32, tag="mm", name="pmask")
            nc.tensor.matmul(pmask[:], ones_sb[:], rcm_r[:, t0:t0 + 264],
                             start=True, stop=True)
            nc.vector.tensor_scalar(rcm_bc[:, t0:t0 + 264], pmask[:], 0.0, None,
                                    op0=OP.add)

        # =========== routing conv1: rc1 = gelu(conv3(x, rc_w1)) ===========
        # computed over buffer cols [56, 584) -> rc1 col c == buffer col 56+c
        RC_LO, RC_W = 56, 528
        with tc.tile_pool(name=R + "routing", bufs=1) as rpool:
            rcw2_sb = {}
            for tap in range(3):
                for k in range(4):
                    t = rpool.tile([128, E], F32R, tag=f"rcw2_{tap}_{k}",
                                   name=f"rcw2_sb{tap}_{k}")
                    nc.sync.dma_start(t[:], rcw2t[tap, k * 128:(k + 1) * 128, :])
                    rcw2_sb[tap, k] = t

            rc1_sb = []
            for m in range(4):
                t = rpool.tile([128, RC_W], F32R, tag=f"rc1_{m}", name=f"rc1_sb{m}")
                rc1_sb.append(t)
            for t0, tw in ((0, 264), (264, 264)):
                plist = [psum.tile([128, tw], F32, tag="mm", name=f"prc_{t0}_{i}")
                         for i in range(4)]
                for tap in range(3):
                    for k in range(4):
                        wsub = wstream.tile([128, 512], F32R, tag="rcw", name="rcwsub")
                        nc.sync.dma_start(wsub[:], rcw1t[tap, k * 128:(k + 1) * 128, :])
                        for m in range(4):
                            nc.tensor.matmul(
                                plist[m][:], wsub[:, m * 128:(m + 1) * 128],
                                xt_sb[k][:, RC_LO + t0 + tap - 1: RC_LO + t0 + tap - 1 + tw],
                                start=(tap == 0 and k == 0), stop=(tap == 2 and k == 3))
                for m in range(4):
                    nc.scalar.activation(rc1_sb[m][:, t0:t0 + tw], plist[m][:], AF.Gelu,
                                         bias=rcb1_sb[:, m:m + 1])
                    nc.vector.tensor_tensor(rc1_sb[m][:, t0:t0 + tw],
                                            rc1_sb[m][:, t0:t0 + tw],
                                            rcm_bc[:, t0:t0 + tw], op=OP.mult)

            # ======= routing conv2 -> logits [E, 512] (center tokens) =======
            lgp = psum.tile([E, TPC], F32, tag="mm", name="lgp")
            first = True
            for tap in range(3):
                for k in range(4):
                    nc.tensor.matmul(
                        lgp[:], rcw2_sb[tap, k][:],
                        rc1_sb[k][:, (C0 - RC_LO) + tap - 1: (C0 - RC_LO) + tap - 1 + TPC],
                        start=first, stop=(tap == 2 and k == 3))
                    first = False
            lg_sb = rpool.tile([E, TPC], F32, tag="lg")
            nc.scalar.activation(lg_sb[:], lgp[:], AF.Identity, bias=rcb2_sb[:])
            if DEBUG[0]:
                nc.sync.dma_start(dbg_lg[:], lg_sb[:])

            # ======= top-2 gating -> gatesT [E, 512] (f32r) =======
            gatesT = acts.tile([E, TPC], F32R)
            for tt in range(4):
                tp = psum.tile([128, E], F32, tag="mm", name="tp")
                nc.tensor.transpose(tp[:], lg_sb[:, tt * 128:(tt + 1) * 128],
                                    id_sb[0:E, 0:E])
                lT = scratch.tile([128, E], F32, tag="lT")
                nc.scalar.copy(lT[:], tp[:])
                m1 = scratch.tile([128, 1], F32, tag="m1")
                nc.vector.reduce_max(m1[:], lT[:], axis=AX.X)
                mask1 = scratch.tile([128, E], F32, tag="mask1")
                nc.vector.tensor_scalar(mask1[:], lT[:], m1[:], None, op0=OP.is_equal)
                lmask = scratch.tile([128, E], F32, tag="lmask")
                nc.vector.scalar_tensor_tensor(lmask[:], mask1[:], -1e30, lT[:],
                                               op0=OP.mult, op1=OP.add)
                m2 = scratch.tile([128, 1], F32, tag="m2")
                nc.vector.reduce_max(m2[:], lmask[:], axis=AX.X)
                mask2 = scratch.tile([128, E], F32, tag="mask2")
                nc.vector.tensor_scalar(mask2[:], lmask[:], m2[:], None, op0=OP.is_equal)
                d = scratch.tile([128, 1], F32, tag="d")
                nc.vector.tensor_scalar(d[:], m2[:], m1[:], None, op0=OP.subtract)
                e_ = scratch.tile([128, 1], F32, tag="e_")
                nc.scalar.activation(e_[:], d[:], AF.Exp)
                ope = scratch.tile([128, 1], F32, tag="ope")
                nc.vector.tensor_scalar(ope[:], e_[:], 1.0, None, op0=OP.add)
                g1 = scratch.tile([128, 1], F32, tag="g1")
                nc.vector.reciprocal(g1[:], ope[:])
                g2 = scratch.tile([128, 1], F32, tag="g2")
                nc.vector.tensor_scalar(g2[:], g1[:], -1.0, 1.0, op0=OP.mult, op1=OP.add)
                t1 = scratch.tile([128, E], F32, tag="t1")
                nc.vector.tensor_scalar(t1[:], mask1[:], g1[:], None, op0=OP.mult)
                gt = scratch.tile([128, E], F32, tag="gt")
                nc.vector.scalar_tensor_tensor(gt[:], mask2[:], g2[:], t1[:],
                                               op0=OP.mult, op1=OP.add)
                gp = psum.tile([E, 128], F32, tag="mm", name="gp")
                nc.tensor.transpose(gp[:], gt[:], id_sb[:])
                nc.vector.tensor_scalar(gatesT[:, tt * 128:(tt + 1) * 128], gp[:],
                                        0.0, None, op0=OP.add)

        if DEBUG[0]:
            gf = acts.tile([E, TPC], F32, tag="gdbg")
            nc.vector.tensor_scalar(gf[:], gatesT[:].bitcast(F32), 0.0, None, op0=OP.add)
            nc.sync.dma_start(dbg_g[:], gf[:])

        # =========== shared conv1: fp8 DoubleRow 3-term compensated ===========
        # sh1 = silu(conv9(x)) -> bf16 [128, 16, 512]
        sh1_sb = acts.tile([128, 16, TPC], BF16, tag="sh1")
        for blk in range(2):
            plist = [psum.tile([128, TPC], F32, tag="mm", name=f"psh_{blk}_{i}")
                     for i in range(8)]
            for tap in range(9):
                for kp in range(2):
                    whi = wstream.tile([128, 2, 1024], F8, tag="whi", name="whi")
                    nc.sync.dma_start(whi[:], shw1h[tap, kp, blk])
                    wlo = wstream.tile([128, 2, 1024], F8L, tag="wlo", name="wlo")
                    nc.sync.dma_start(wlo[:], shw1l[tap, kp, blk])
                    first = (tap == 0 and kp == 0)
                    last = (tap == 8 and kp == 1)
                    for mi in range(8):
                        ws = slice(mi * 128, (mi + 1) * 128)
                        xs_h = x8h_sb[:, 2 * kp:2 * kp + 2, tap:tap + TPC]
                        xs_l = x8l_sb[:, 2 * kp:2 * kp + 2, tap:tap + TPC]
                        nc.tensor.matmul(plist[mi][:], whi[:, :, ws], xs_h,
                                         start=first, stop=False, perf_mode=DR)
                        nc.tensor.matmul(plist[mi][:], whi[:, :, ws], xs_l,
                                         start=False, stop=False, perf_mode=DR)
                        nc.tensor.matmul(plist[mi][:], wlo[:, :, ws], xs_h,
                                         start=False, stop=(last and True), perf_mode=DR)
            for mi in range(8):
                m = blk * 8 + mi
                nc.scalar.activation(sh1_sb[:, m, :], plist[mi][:], AF.Silu,
                                     bias=shb1_sb[:, m:m + 1])

        # =========== shared conv2 (k=1, bf16): sh2 [4][128, 512] f32 ===========
        sh2_sb = []
        for mo in range(4):
            t = acts.tile([128, TPC], F32, tag=f"sh2_{mo}", name=f"sh2_sb{mo}")
            sh2_sb.append(t)
        s2list = [psum.tile([128, TPC], F32, tag="mm", name=f"ps2_{i}") for i in range(4)]
        for g in range(4):
            w2t = wstream.tile([128, 4, D_IN], BF16, tag="w2", name="w2t")
            nc.sync.dma_start(w2t[:], shw2p[g])
            for kk in range(4):
                k = g * 4 + kk
                for mo in range(4):
                    nc.tensor.matmul(s2list[mo][:], w2t[:, kk, mo * 128:(mo + 1) * 128],
                                     sh1_sb[:, k, :], start=(k == 0), stop=(k == 15))
        for mo in range(4):
            nc.scalar.activation(sh2_sb[mo][:], s2list[mo][:], AF.Identity,
                                 bias=shb2_sb[:, mo:mo + 1])
        if DEBUG[0]:
            nc.sync.dma_start(dbg_sh1[:], sh1_sb[:])
            nc.sync.dma_start(dbg_sh2[:], sh2_sb[0][:])

        # =========== experts (dense fp8 DoubleRow, gated accumulate) ===========
        y_acc = [acts.tile([128, TPC], F32, tag=f"y{mo}", name=f"y_acc_{mo}")
                 for mo in range(4)]
        for e in range(E):
            b1 = scratch.tile([128, 16], F32, tag="b1")
            nc.sync.dma_start(b1[:], eb1p[e])
            bce = scratch.tile([128, 4], F32, tag="bce")
            nc.sync.dma_start(bce[:], ebcp[e])
            w1t = epool.tile([128, 2, 2, D_HID], F8, tag="w1", name="w1t")
            nc.sync.dma_start(w1t[:], ew1p[e])  # dram [p, kp, j, m] matches tile order
            w2e = epool.tile([128, 8, 2, D_IN], F8, tag="w2e", name="w2e")
            nc.sync.dma_start(w2e[:], ew2p[e])

            h8_sb = epool.tile([128, 8, 2, TPC], F8, tag="h8", name="h8_sb")
            xc_h = x8h_sb[:, :, H8:H8 + TPC]
            for mb in range(4):
                plist = [psum.tile([128, TPC], F32, tag="mm", name=f"ph_{e}_{mb}_{i}")
                         for i in range(4)]
                for kp in range(2):
                    for mi in range(4):
                        m = mb * 4 + mi
                        nc.tensor.matmul(
                            plist[mi][:],
                            w1t[:, kp, :, m * 128:(m + 1) * 128],
                            x8h_sb[:, 2 * kp:2 * kp + 2, H8:H8 + TPC],
                            start=(kp == 0), stop=(kp == 1), perf_mode=DR)
                for mi in range(4):
                    m = mb * 4 + mi
                    # elu(v) = max(v, min(exp(v), 1) - 1), v = h + b1
                    eh = scratch.tile([128, TPC], F32, tag="he")
                    nc.scalar.activation(eh[:], plist[mi][:], AF.Exp,
                                         bias=b1[:, m:m + 1])
                    em = scratch.tile([128, TPC], F32, tag="hm")
                    nc.gpsimd.tensor_scalar(em[:], eh[:], 1.0, -1.0,
                                            op0=OP.min, op1=OP.add)
                    nc.vector.scalar_tensor_tensor(h8_sb[:, m // 2, m % 2, :],
                                                   plist[mi][:],
                                                   b1[:, m:m + 1], em[:],
                                                   op0=OP.add, op1=OP.max)

            # broadcast this expert's gate row
            bcp = psum.tile([128, TPC], F32, tag="mm", name="bcp")
            nc.tensor.matmul(bcp[:], sel_sb[:, e * 128:(e + 1) * 128], gatesT[:],
                             start=True, stop=True)
            bc_sb = scratch.tile([128, TPC], F32, tag="bc")
            nc.scalar.copy(bc_sb[:], bcp[:])

            if DEBUG[0] and e == 0:
                nc.sync.dma_start(dbg_h8[:], h8_sb[:])
            elist = [psum.tile([128, TPC], F32, tag="mm", name=f"pe_{e}_{i}")
                     for i in range(4)]
            for kp in range(8):
                for mo in range(4):
                    nc.tensor.matmul(elist[mo][:],
                                     w2e[:, kp, :, mo * 128:(mo + 1) * 128],
                                     h8_sb[:, kp, :, :],
                                     start=(kp == 0), stop=(kp == 7), perf_mode=DR)
            for mo in range(4):
                # (eo + e_b2) * gate, accumulated into y_acc
                if e == 0:
                    nc.vector.scalar_tensor_tensor(
                        y_acc[mo][:], elist[mo][:], bce[:, mo:mo + 1], bc_sb[:],
                        op0=OP.add, op1=OP.mult)
                else:
                    yt = scratch.tile([128, TPC], F32, tag="yt")
                    nc.vector.scalar_tensor_tensor(
                        yt[:], elist[mo][:], bce[:, mo:mo + 1], bc_sb[:],
                        op0=OP.add, op1=OP.mult)
                    nc.gpsimd.tensor_tensor(y_acc[mo][:], y_acc[mo][:], yt[:],
                                            op=OP.add)

        if DEBUG[0]:
            nc.sync.dma_start(dbg_y[:], y_acc[0][:])

        # =========== z = x + y + sh2 ; transpose ; layernorm ; out ===========
        z_sb = []
        for mo in range(4):
            z = acts.tile([128, TPC], F32, tag=f"z{mo}", name=f"z_sb{mo}")
            nc.vector.tensor_tensor(z[:], y_acc[mo][:], sh2_sb[mo][:], op=OP.add)
            nc.vector.tensor_tensor(z[:], z[:],
                                    xt_sb[mo][:, C0:C0 + TPC].bitcast(F32), op=OP.add)
            z_sb.append(z)

        for tt in range(4):
            zT = scratch.tile([128, D_IN], F32, tag="zT")
            for mo in range(4):
                ztp = psum.tile([128, 128], F32, tag="mm", name="ztp")
                nc.tensor.transpose(ztp[:], z_sb[mo][:, tt * 128:(tt + 1) * 128],
                                    id_sb[:])
                nc.scalar.copy(zT[:, mo * 128:(mo + 1) * 128], ztp[:])
            srow = scratch.tile([128, 1], F32, tag="srow")
            nc.vector.reduce_sum(srow[:], zT[:], axis=AX.X)
            nmean = scratch.tile([128, 1], F32, tag="nmean")
            nc.vector.tensor_scalar(nmean[:], srow[:], -1.0 / D_IN, None, op0=OP.mult)
            zc = scratch.tile([128, D_IN], F32, tag="zc")
            nc.vector.tensor_scalar(zc[:], zT[:], nmean[:], None, op0=OP.add)
            sq = scratch.tile([128, D_IN], F32, tag="sq")
            ssq = scratch.tile([128, 1], F32, tag="ssq")
            nc.scalar.activation(sq[:], zc[:], AF.Square, accum_out=ssq[:])
            vpe = scratch.tile([128, 1], F32, tag="vpe")
            nc.vector.tensor_scalar(vpe[:], ssq[:], 1.0 / D_IN, 1e-5,
                                    op0=OP.mult, op1=OP.add)
            rinv = scratch.tile([128, 1], F32, tag="rinv")
            nc.vector.reciprocal(rinv[:], vpe[:])
            rstd = scratch.tile([128, 1], F32, tag="rstd")
            nc.scalar.activation(rstd[:], rinv[:], AF.Sqrt)
            normed = scratch.tile([128, D_IN], F32, tag="normed")
            nc.vector.tensor_scalar(normed[:], zc[:], rstd[:], None, op0=OP.mult)
            og = scratch.tile([128, D_IN], F32, tag="og")
            nc.vector.tensor_tensor(og[:], normed[:], lng_bc[:], op=OP.mult)
            out = scratch.tile([128, D_IN], F32, tag="out")
            nc.vector.tensor_tensor(out[:], og[:], lnb_bc[:], op=OP.add)
            nc.sync.dma_start(yout[tt * 128:(tt + 1) * 128, :], out[:])

    _split_multi_waits(nc)
    return nc


_CACHE = {}


def _get_nc(reps=1):
    key = f"nc{reps}"
    if key not in _CACHE:
        _CACHE[key] = _build(reps)
    return _CACHE[key]


LAST_RESULT = {}


def kernel(x, rc_w1, rc_b1, rc_w2, rc_b2, sh_w1, sh_b1, sh_w2, sh_b2,
           e_w1, e_b1, e_w2, e_b2, ln_g, ln_b, **kwargs):
    import ml_dtypes
    FP8 = ml_dtypes.float8_e4m3
    FP8L = ml_dtypes.float8_e5m2
    BF = ml_dtypes.bfloat16

    x = np.asarray(x, np.float32)
    f = lambda a: np.ascontiguousarray(np.asarray(a, np.float32))

    def q8(a):
        return np.asarray(a, np.float32).astype(FP8)

    def qe5(a):
        return np.asarray(a, np.float32).astype(FP8L)

    # shared conv1: hi/lo packs [tap, kp, blk, p, j, mcol]
    w1 = np.asarray(sh_w1, np.float32)            # [2048, 512, 9]
    w1t = w1.transpose(2, 1, 0)                   # [9, 512, 2048]
    w1h = q8(w1t)
    w1l = qe5(w1t - w1h.astype(np.float32))
    def packw1(a):
        # [9, 512, 2048] -> [9, kp2, blk2, p128, j2, 1024]
        a = a.reshape(9, 2, 2, 128, 2, 1024)      # [tap, kp, j, p, blk, mcol]
        return np.ascontiguousarray(a.transpose(0, 1, 4, 3, 2, 5))
    shw1h_a = packw1(w1h)
    shw1l_a = packw1(w1l)

    # shared conv2 bf16: [g, p, kk, d]
    w2 = np.asarray(sh_w2, np.float32)[:, :, 0]   # [512, 2048]
    shw2p_a = np.ascontiguousarray(
        w2.T.reshape(4, 4, 128, 512).transpose(0, 2, 1, 3)).astype(BF)

    # expert weights fp8 DoubleRow packs: [e, p, kp, j, m]
    ew1 = q8(e_w1)                                # [E, 512, 2048]
    ew1p_a = np.ascontiguousarray(
        ew1.reshape(E, 2, 2, 128, D_HID).transpose(0, 3, 1, 2, 4))
    ew2 = q8(e_w2)                                # [E, 2048, 512]
    ew2p_a = np.ascontiguousarray(
        ew2.reshape(E, 8, 2, 128, D_IN).transpose(0, 3, 1, 2, 4))

    shared = {
        "rcw1t": f(np.asarray(rc_w1, np.float32).transpose(2, 1, 0)),
        "rcw2t": f(np.asarray(rc_w2, np.float32).transpose(2, 1, 0)),
        "shw1h": shw1h_a,
        "shw1l": shw1l_a,
        "shw2p": shw2p_a,
        "ew1p": ew1p_a,
        "ew2p": ew2p_a,
        "rcb1": f(np.asarray(rc_b1, np.float32).reshape(4, 128).T),
        "rcb2": f(np.asarray(rc_b2, np.float32).reshape(E, 1)),
        "shb1": f(np.asarray(sh_b1, np.float32).reshape(16, 128).T),
        "shb2": f(np.asarray(sh_b2, np.float32).reshape(4, 128).T),
        "eb1p": f(np.asarray(e_b1, np.float32).reshape(E, 16, 128).transpose(0, 2, 1)),
        "ebcp": f(np.asarray(e_b2, np.float32).reshape(E, 4, 128).transpose(0, 2, 1)),
        "lng": f(np.asarray(ln_g, np.float32).reshape(1, D_IN)),
        "lnb": f(np.asarray(ln_b, np.float32).reshape(1, D_IN)),
        "ones": np.ones((1, 128), np.float32),
        "sel": np.repeat(np.eye(E, dtype=np.float32), 128, axis=1),
        "ident": np.eye(128, dtype=np.float32),
    }

    in_maps = []
    for c in range(NCORES):
        b, j = divmod(c, T // TPC)
        # f32r slab with HALO-wide halo
        lo_tok = j * TPC - HALO
        hi_tok = j * TPC + TPC + HALO
        xh = np.zeros((W, D_IN), np.float32)
        lo = max(0, lo_tok)
        hi = min(T, hi_tok)
        xh[lo - lo_tok: hi - lo_tok] = x[b, lo:hi]
        im = dict(shared)
        im["xt"] = np.ascontiguousarray(xh.T)
        # fp8 slab with H8-wide halo: [p, k, col]
        lo8, hi8 = j * TPC - H8, j * TPC + TPC + H8
        xh8 = np.zeros((W8, D_IN), np.float32)
        lo_c, hi_c = max(0, lo8), min(T, hi8)
        xh8[lo_c - lo8: hi_c - lo8] = x[b, lo_c:hi_c]
        xthi = q8(xh8.T)                          # [512, 520]
        xtlo = qe5(xh8.T - xthi.astype(np.float32))
        im["x8h"] = np.ascontiguousarray(xthi.reshape(4, 128, W8).transpose(1, 0, 2))
        im["x8l"] = np.ascontiguousarray(xtlo.reshape(4, 128, W8).transpose(1, 0, 2))
        rcm = np.zeros((1, 528), np.float32)
        for cidx in range(528):
            tok = j * TPC - 8 + cidx
            if 0 <= tok < T:
                rcm[0, cidx] = 1.0
        im["rcmask"] = rcm
        in_maps.append(im)

    nc = _get_nc()
    res = run_bass_kernel_spmd(nc, in_maps, core_ids=list(range(NCORES)),
                               **kwargs)
    LAST_RESULT["res"] = res

    out = np.empty((B, T, D_IN), np.float32)
    for c in range(NCORES):
        b, j = divmod(c, T // TPC)
        out[b, j * TPC:(j + 1) * TPC] = res.results[c]["yout"]
    return out


# revision 39
# speedup vs baseline: 1.6127x; 1.2930x over previous
"""Trainium2 Bass kernel for nn_MixtureOfExperts_29867202576447.

Strategy: data-parallel over tokens (8 cores x 512 tokens).
 - Routing convs + top-2 gating: float32r (exact top-k selection).
 - Shared conv1 (9-tap, 512->2048): fp8 DoubleRow matmuls with 3-term
   error compensation (e4m3 hi values + e5m2 residuals for both x and W),
   all three terms accumulated in one PSUM bank. SiLU -> bf16.
 - Shared conv2 (1x1, 2048->512): bf16.
 - Experts (10x dense, gated): plain fp8e4m3 DoubleRow for both GEMMs.
 - Final residual + layernorm in f32.

Self-contained: hardcodes all shapes; host-side prep shards/pads x,
quantizes + packs weights for the DoubleRow K-pair layout.
"""
import numpy as np
from contextlib import ExitStack

import concourse.bass as bass
import concourse.tile as tile
import concourse.mybir as mybir
from concourse.bass_utils import run_bass_kernel_spmd

F32R = mybir.dt.float32r
F32 = mybir.dt.float32
BF16 = mybir.dt.bfloat16
F8 = mybir.dt.float8e4
F8L = mybir.dt.float8e5
AF = mybir.ActivationFunctionType
OP = mybir.AluOpType
AX = mybir.AxisListType
DR = mybir.MatmulPerfMode.DoubleRow

D_IN, D_HID, E = 512, 2048, 10
B, T = 2, 2048
TPC = 512          # tokens per core
HALO = 64          # halo columns each side of the f32r token window
W = TPC + 2 * HALO  # 640 buffer columns (routing/residual slab)
H8 = 4             # halo of the fp8 slab (9-tap conv)
W8 = TPC + 2 * H8  # 520 columns
NCORES = 8

_ctr = [0]


def _split_multi_waits(nc, max_waits=1):
    """walrus accepts one sync-wait per instruction; hoist extras onto
    same-engine NoOps placed immediately before the instruction."""
    n = 0
    for f in nc.m.functions:
        for bb in f.blocks:
            out = []
            changed = False
            for ins in bb.instructions:
                si = getattr(ins, "sync_info", None)
                waits = list(si.on_wait) if (si is not None and si.on_wait) else []
                if len(waits) > max_waits:
                    for w in waits[:-max_waits]:
                        _ctr[0] += 1
                        nop = mybir.InstNoOp(
                            name=f"I-waitsplit-{_ctr[0]}", engine=ins.engine,
                            ins=[], outs=[])
                        nop.sync_info = mybir.SyncInfo(on_wait=[w], on_update=[])
                        nc.register_instruction(nop)
                        out.append(nop)
                    si.on_wait = waits[-max_waits:]
                    changed = True
                    n += 1
                out.append(ins)
            if changed:
                bb.instructions = out
    return n


DEBUG = [False]


def _build(reps=1):
    nc = bass.Bass(trn_type="TRN2")

    # ---------------- DRAM I/O ----------------
    xt = nc.dram_tensor("xt", [D_IN, W], F32R, kind="ExternalInput")
    x8h = nc.dram_tensor("x8h", [128, 4, W8], F8, kind="ExternalInput")
    x8l = nc.dram_tensor("x8l", [128, 4, W8], F8L, kind="ExternalInput")
    rcw1t = nc.dram_tensor("rcw1t", [3, D_IN, D_IN], F32R, kind="ExternalInput")
    rcw2t = nc.dram_tensor("rcw2t", [3, D_IN, E], F32R, kind="ExternalInput")
    # shared conv1 weights: [tap, kp, p, j, m] (full m; DMA'd in m-block slices)
    shw1h = nc.dram_tensor("shw1h", [9, 2, 128, 2, 2048], F8, kind="ExternalInput")
    shw1l = nc.dram_tensor("shw1l", [9, 2, 128, 2, 2048], F8L, kind="ExternalInput")
    # shared conv2 weights bf16: [g, p, kk, d]
    shw2p = nc.dram_tensor("shw2p", [4, 128, 4, D_IN], BF16, kind="ExternalInput")
    # expert weights fp8, DoubleRow K-pair layout
    ew1p = nc.dram_tensor("ew1p", [E, 128, 2, 2, D_HID], F8, kind="ExternalInput")
    ew2p = nc.dram_tensor("ew2p", [E, 128, 8, 2, D_IN], F8, kind="ExternalInput")
    rcb1 = nc.dram_tensor("rcb1", [128, 4], F32, kind="ExternalInput")
    rcb2 = nc.dram_tensor("rcb2", [E, 1], F32, kind="ExternalInput")
    shb1 = nc.dram_tensor("shb1", [128, 16], F32, kind="ExternalInput")
    shb2 = nc.dram_tensor("shb2", [128, 4], F32, kind="ExternalInput")
    eb1p = nc.dram_tensor("eb1p", [E, 128, 16], F32, kind="ExternalInput")
    ebcp = nc.dram_tensor("ebcp", [E, 128, 4], F32, kind="ExternalInput")
    lng = nc.dram_tensor("lng", [1, D_IN], F32R, kind="ExternalInput")
    lnb = nc.dram_tensor("lnb", [1, D_IN], F32R, kind="ExternalInput")
    ones = nc.dram_tensor("ones", [1, 128], F32R, kind="ExternalInput")
    sel = nc.dram_tensor("sel", [E, E * 128], F32R, kind="ExternalInput")
    ident = nc.dram_tensor("ident", [128, 128], F32, kind="ExternalInput")
    rcmask = nc.dram_tensor("rcmask", [1, 528], F32R, kind="ExternalInput")
    onesc = nc.dram_tensor("onesc", [128, 1], F32R, kind="ExternalInput")
    lngc = nc.dram_tensor("lngc", [128, 4], F32, kind="ExternalInput")
    lnbc2 = nc.dram_tensor("lnbc2", [128, 4], F32, kind="ExternalInput")
    yout = nc.dram_tensor("yout", [D_IN, TPC], F32, kind="ExternalOutput")
    if DEBUG[0]:
        dbg_lg = nc.dram_tensor("dbg_lg", [E, TPC], F32, kind="ExternalOutput")
        dbg_g = nc.dram_tensor("dbg_g", [E, TPC], F32, kind="ExternalOutput")
        dbg_sh1 = nc.dram_tensor("dbg_sh1", [128, 16, TPC], BF16, kind="ExternalOutput")
        dbg_sh2 = nc.dram_tensor("dbg_sh2", [128, TPC], F32, kind="ExternalOutput")
        dbg_y = nc.dram_tensor("dbg_y", [128, TPC], F32, kind="ExternalOutput")
        dbg_h8 = nc.dram_tensor("dbg_h8", [128, 8, 2, TPC], F8, kind="ExternalOutput")

    C0 = HALO            # f32r-slab col of first center token
    with tile.TileContext(nc) as tc:
      for rep in range(reps):
       with ExitStack() as ctx:
        R = f"r{rep}_"
        const = ctx.enter_context(tc.tile_pool(name=R + "const", bufs=1))
        acts = ctx.enter_context(tc.tile_pool(name=R + "acts", bufs=1))
        wstream = ctx.enter_context(tc.tile_pool(name=R + "wstream", bufs=3))
        scratch = ctx.enter_context(tc.tile_pool(name=R + "scratch", bufs=2))
        epool = ctx.enter_context(tc.tile_pool(name=R + "epool", bufs=2))
        psum = ctx.enter_context(tc.tile_pool(name=R + "psum", bufs=8, space="PSUM"))

        # ---- x + routing weights interleaved: the rc conv consumes (xt_k,
        # rcw[tap0,k]) pairs first, so land them in that order ----
        RC_LO, RC_W = 56, 528
        rpool = ctx.enter_context(tc.tile_pool(name=R + "routing", bufs=1))
        xt_sb = [None] * 4
        rcw_sb = {}
        for k in range(4):
            t = acts.tile([128, W], F32R, tag=f"xt{k}", name=f"xt_sb{k}")
            nc.sync.dma_start(t[:], xt[k * 128:(k + 1) * 128, :])
            xt_sb[k] = t
            for tap in range(3):
                w = rpool.tile([128, 512], F32R, tag=f"rcw_{tap}_{k}",
                               name=f"rcw_sb{tap}_{k}")
                nc.sync.dma_start(w[:], rcw1t[tap, k * 128:(k + 1) * 128, :])
                rcw_sb[tap, k] = w

        # ---------------- constants ----------------
        id_sb = const.tile([128, 128], F32)
        nc.sync.dma_start(id_sb[:], ident[:])
        x8h_sb = acts.tile([128, 4, W8], F8, tag="x8h")
        nc.sync.dma_start(x8h_sb[:], x8h[:])
        x8l_sb = acts.tile([128, 4, W8], F8L, tag="x8l")
        nc.sync.dma_start(x8l_sb[:], x8l[:])
        ones_sb = const.tile([1, 128], F32R)
        nc.sync.dma_start(ones_sb[:], ones[:])
        sel_sb = const.tile([E, E * 128], F32R)
        nc.sync.dma_start(sel_sb[:], sel[:])
        rcb1_sb = const.tile([128, 4], F32)
        nc.sync.dma_start(rcb1_sb[:], rcb1[:])
        rcb2_sb = const.tile([E, 1], F32)
        nc.sync.dma_start(rcb2_sb[:], rcb2[:])
        shb1_sb = const.tile([128, 16], F32)
        nc.sync.dma_start(shb1_sb[:], shb1[:])
        shb2_sb = const.tile([128, 4], F32)
        nc.sync.dma_start(shb2_sb[:], shb2[:])
        lng_r = const.tile([1, D_IN], F32R)
        nc.sync.dma_start(lng_r[:], lng[:])
        lnb_r = const.tile([1, D_IN], F32R)
        nc.sync.dma_start(lnb_r[:], lnb[:])

        onesc_sb = const.tile([128, 1], F32R)
        nc.sync.dma_start(onesc_sb[:], onesc[:])
        lngc_sb = const.tile([128, 4], F32)
        nc.sync.dma_start(lngc_sb[:], lngc[:])
        lnbc2_sb = const.tile([128, 4], F32)
        nc.sync.dma_start(lnbc2_sb[:], lnbc2[:])

        # rc edge mask: zero rc1 at columns outside the sequence
        rcm_r = const.tile([1, 528], F32R)
        nc.sync.dma_start(rcm_r[:], rcmask[:])
        rcm_bc = const.tile([128, 528], F32R)
        for t0 in (0, 264):
            pmask = psum.tile([128, 264], F32, tag="c1", bufs=4, name="pmask")
            nc.tensor.matmul(pmask[:], ones_sb[:], rcm_r[:, t0:t0 + 264],
                             start=True, stop=True)
            nc.vector.tensor_scalar(rcm_bc[:, t0:t0 + 264], pmask[:], 0.0, None,
                                    op0=OP.add)

        # =========== routing conv1: rc1 = gelu(conv3(x, rc_w1)) ===========
        # computed over buffer cols [56, 584) -> rc1 col c == buffer col 56+c
        gatesT = acts.tile([E, TPC], F32R, name="gatesT")

        def rc_steps():
            rcw2_sb = {}
            for tap in range(3):
                for k in range(4):
                    t = rpool.tile([128, E], F32R, tag=f"rcw2_{tap}_{k}",
                                   name=f"rcw2_sb{tap}_{k}")
                    nc.sync.dma_start(t[:], rcw2t[tap, k * 128:(k + 1) * 128, :])
                    rcw2_sb[tap, k] = t

            rc1_sb = []
            for m in range(4):
                t = rpool.tile([128, RC_W], F32R, tag=f"rc1_{m}", name=f"rc1_sb{m}")
                rc1_sb.append(t)
            for t0, tw in ((0, 264), (264, 264)):
                plist = [psum.tile([128, tw], F32, tag="c1", bufs=4, name=f"prc_{t0}_{i}")
                         for i in range(4)]
                for tap in range(3):
                    for k in range(4):
                        for m in range(4):
                            nc.tensor.matmul(
                                plist[m][:], rcw_sb[tap, k][:, m * 128:(m + 1) * 128],
                                xt_sb[k][:, RC_LO + t0 + tap - 1: RC_LO + t0 + tap - 1 + tw],
                                start=(tap == 0 and k == 0), stop=(tap == 2 and k == 3))
                    yield
                for m in range(4):
                    nc.scalar.activation(rc1_sb[m][:, t0:t0 + tw], plist[m][:], AF.Gelu,
                                         bias=rcb1_sb[:, m:m + 1])
                    nc.vector.tensor_tensor(rc1_sb[m][:, t0:t0 + tw],
                                            rc1_sb[m][:, t0:t0 + tw],
                                            rcm_bc[:, t0:t0 + tw], op=OP.mult)

            # ======= routing conv2 -> logits [E, 512] (center tokens) =======
            lgp = psum.tile([E, TPC], F32, tag="c1", bufs=4, name="lgp")
            first = True
            for tap in range(3):
                for k in range(4):
                    nc.tensor.matmul(
                        lgp[:], rcw2_sb[tap, k][:],
                        rc1_sb[k][:, (C0 - RC_LO) + tap - 1: (C0 - RC_LO) + tap - 1 + TPC],
                        start=first, stop=(tap == 2 and k == 3))
                    first = False
            lg_sb = rpool.tile([E, TPC], F32, tag="lg")
            nc.scalar.activation(lg_sb[:], lgp[:], AF.Identity, bias=rcb2_sb[:])
            if DEBUG[0]:
                nc.sync.dma_start(dbg_lg[:], lg_sb[:])
            yield

            # ======= top-2 gating -> gatesT [E, 512] (f32r) =======
            for tt in range(4):
                tp = psum.tile([128, E], F32, tag="c1", bufs=4, name="tp")
                nc.tensor.transpose(tp[:], lg_sb[:, tt * 128:(tt + 1) * 128],
                                    id_sb[0:E, 0:E])
                lT = scratch.tile([128, E], F32, tag="lT")
                nc.scalar.copy(lT[:], tp[:])
                m1 = scratch.tile([128, 1], F32, tag="m1")
                nc.vector.reduce_max(m1[:], lT[:], axis=AX.X)
                mask1 = scratch.tile([128, E], F32, tag="mask1")
                nc.vector.tensor_scalar(mask1[:], lT[:], m1[:], None, op0=OP.is_equal)
                lmask = scratch.tile([128, E], F32, tag="lmask")
                nc.vector.scalar_tensor_tensor(lmask[:], mask1[:], -1e30, lT[:],
                                               op0=OP.mult, op1=OP.add)
                m2 = scratch.tile([128, 1], F32, tag="m2")
                nc.vector.reduce_max(m2[:], lmask[:], axis=AX.X)
                mask2 = scratch.tile([128, E], F32, tag="mask2")
                nc.vector.tensor_scalar(mask2[:], lmask[:], m2[:], None, op0=OP.is_equal)
                d = scratch.tile([128, 1], F32, tag="d")
                nc.vector.tensor_scalar(d[:], m2[:], m1[:], None, op0=OP.subtract)
                e_ = scratch.tile([128, 1], F32, tag="e_")
                nc.scalar.activation(e_[:], d[:], AF.Exp)
                ope = scratch.tile([128, 1], F32, tag="ope")
                nc.vector.tensor_scalar(ope[:], e_[:], 1.0, None, op0=OP.add)
                g1 = scratch.tile([128, 1], F32, tag="g1")
                nc.vector.reciprocal(g1[:], ope[:])
                g2 = scratch.tile([128, 1], F32, tag="g2")
                nc.vector.tensor_scalar(g2[:], g1[:], -1.0, 1.0, op0=OP.mult, op1=OP.add)
                t1 = scratch.tile([128, E], F32, tag="t1")
                nc.vector.tensor_scalar(t1[:], mask1[:], g1[:], None, op0=OP.mult)
                gt = scratch.tile([128, E], F32, tag="gt")
                nc.vector.scalar_tensor_tensor(gt[:], mask2[:], g2[:], t1[:],
                                               op0=OP.mult, op1=OP.add)
                gp = psum.tile([E, 128], F32, tag="c1", bufs=4, name="gp")
                nc.tensor.transpose(gp[:], gt[:], id_sb[:])
                nc.vector.tensor_scalar(gatesT[:, tt * 128:(tt + 1) * 128], gp[:],
                                        0.0, None, op0=OP.add)
                yield

        def dbg_gates():
            if DEBUG[0]:
                gf = acts.tile([E, TPC], F32, tag="gdbg")
                nc.vector.tensor_scalar(gf[:], gatesT[:].bitcast(F32), 0.0, None, op0=OP.add)
                nc.sync.dma_start(dbg_g[:], gf[:])
            yield

        # ==== shared conv1 (fp8 DoubleRow 3-term) interleaved with experts ====
        # conv1 emits in (blk, tap, kp) steps using 6 psum banks; expert work
        # is emitted as small chunks between steps so PE/Act/Pool/DVE overlap.
        sh1_sb = acts.tile([128, 16, TPC], BF16, tag="sh1")
        y_acc = [acts.tile([128, TPC], F32, tag=f"y{mo}", name=f"y_acc_{mo}")
                 for mo in range(4)]
        sh2_sb = []
        for mo in range(4):
            t = acts.tile([128, TPC], F32, tag=f"sh2_{mo}", name=f"sh2_sb{mo}")
            sh2_sb.append(t)

        C1_BLOCKS = ((0, 4), (4, 4), (8, 4), (12, 4))

        def conv1_steps():
            for m0, mw in C1_BLOCKS:
                plist = [psum.tile([128, TPC], F32, tag="c1", bufs=4,
                                   name=f"psh_{m0}_{i}") for i in range(mw)]
                for tap in range(9):
                    for kp in range(2):
                        whi = wstream.tile([128, 2, mw * 128], F8, tag="whi",
                                           name="whi", bufs=3)
                        nc.sync.dma_start(
                            whi[:], shw1h[tap, kp, :, :, m0 * 128:(m0 + mw) * 128])
                        wlo = wstream.tile([128, 2, mw * 128], F8L, tag="wlo",
                                           name="wlo", bufs=3)
                        nc.sync.dma_start(
                            wlo[:], shw1l[tap, kp, :, :, m0 * 128:(m0 + mw) * 128])
                        first = (tap == 0 and kp == 0)
                        last = (tap == 8 and kp == 1)
                        xs_h = x8h_sb[:, 2 * kp:2 * kp + 2, tap:tap + TPC]
                        xs_l = x8l_sb[:, 2 * kp:2 * kp + 2, tap:tap + TPC]
                        for mi in range(mw):
                            ws = slice(mi * 128, (mi + 1) * 128)
                            nc.tensor.matmul(plist[mi][:], whi[:, :, ws], xs_h,
                                             start=first, stop=False, perf_mode=DR)
                            nc.tensor.matmul(plist[mi][:], whi[:, :, ws], xs_l,
                                             start=False, stop=False, perf_mode=DR)
                            nc.tensor.matmul(plist[mi][:], wlo[:, :, ws], xs_h,
                                             start=False, stop=last, perf_mode=DR)
                        yield
                for mi in range(mw):
                    m = m0 + mi
                    nc.scalar.activation(sh1_sb[:, m, :], plist[mi][:], AF.Silu,
                                         bias=shb1_sb[:, m:m + 1])
                yield

        def conv2_steps():
            s2list = [psum.tile([128, TPC], F32, tag="c1", bufs=4,
                                name=f"ps2_{i}") for i in range(4)]
            for g in range(4):
                w2t = wstream.tile([128, 4, D_IN], BF16, tag="w2", name="w2t",
                                   bufs=2)
                nc.sync.dma_start(w2t[:], shw2p[g])
                for kk in range(4):
                    k = g * 4 + kk
                    for mo in range(4):
                        nc.tensor.matmul(s2list[mo][:],
                                         w2t[:, kk, mo * 128:(mo + 1) * 128],
                                         sh1_sb[:, k, :],
                                         start=(k == 0), stop=(k == 15))
                    yield
            for mo in range(4):
                nc.scalar.activation(sh2_sb[mo][:], s2list[mo][:], AF.Identity,
                                     bias=shb2_sb[:, mo:mo + 1])
            if DEBUG[0]:
                nc.sync.dma_start(dbg_sh1[:], sh1_sb[:])
                nc.sync.dma_start(dbg_sh2[:], sh2_sb[0][:])
            yield

        def eload(e):
            b1 = scratch.tile([128, 16], F32, tag="b1", name="b1")
            nc.sync.dma_start(b1[:], eb1p[e])
            bce = scratch.tile([128, 4], F32, tag="bce", name="bce")
            nc.sync.dma_start(bce[:], ebcp[e])
            w1t = epool.tile([128, 2, 2, D_HID], F8, tag="w1", name="w1t")
            nc.sync.dma_start(w1t[:], ew1p[e])
            w2e = epool.tile([128, 8, 2, D_IN], F8, tag="w2e", name="w2e")
            nc.sync.dma_start(w2e[:], ew2p[e])
            return b1, bce, w1t, w2e

        def expert_chunks():
            nxt = eload(0)
            for e in range(E):
                b1, bce, w1t, w2e = nxt
                if e + 1 < E:
                    nxt = eload(e + 1)
                h8_sb = epool.tile([128, 8, 2, TPC], F8, tag="h8", name="h8_sb")

                for m in range(16):          # gemm1, one m-tile per chunk
                    ph = psum.tile([128, TPC], F32, tag="mm", bufs=4,
                                   name=f"ph_{e}_{m}")
                    for kp in range(2):
                        nc.tensor.matmul(
                            ph[:],
                            w1t[:, kp, :, m * 128:(m + 1) * 128],
                            x8h_sb[:, 2 * kp:2 * kp + 2, H8:H8 + TPC],
                            start=(kp == 0), stop=(kp == 1), perf_mode=DR)
                    # elu(v) = max(v, min(exp(v), 1) - 1), v = h + b1
                    eh = scratch.tile([128, TPC], F32, tag="he")
                    nc.scalar.activation(eh[:], ph[:], AF.Exp,
                                         bias=b1[:, m:m + 1])
                    em = scratch.tile([128, TPC], F32, tag="hm")
                    nc.gpsimd.tensor_scalar(em[:], eh[:], 1.0, -1.0,
                                            op0=OP.min, op1=OP.add)
                    nc.vector.scalar_tensor_tensor(h8_sb[:, m // 2, m % 2, :],
                                                   ph[:],
                                                   b1[:, m:m + 1], em[:],
                                                   op0=OP.add, op1=OP.max)
                    yield

                # broadcast this expert's gate row
                bcp = psum.tile([128, TPC], F32, tag="mm", bufs=4, name="bcp")
                nc.tensor.matmul(bcp[:], sel_sb[:, e * 128:(e + 1) * 128],
                                 gatesT[:], start=True, stop=True)
                bc_sb = scratch.tile([128, TPC], F32, tag="bc")
                nc.scalar.copy(bc_sb[:], bcp[:])
                if DEBUG[0] and e == 0:
                    nc.sync.dma_start(dbg_h8[:], h8_sb[:])
                yield

                for mo in range(4):          # gemm2, one mo-tile per chunk
                    pe_ = psum.tile([128, TPC], F32, tag="mm", bufs=4,
                                    name=f"pe_{e}_{mo}")
                    for kp in range(8):
                        nc.tensor.matmul(pe_[:],
                                         w2e[:, kp, :, mo * 128:(mo + 1) * 128],
                                         h8_sb[:, kp, :, :],
                                         start=(kp == 0), stop=(kp == 7),
                                         perf_mode=DR)
                    # (eo + e_b2) * gate, accumulated into y_acc
                    if e == 0:
                        nc.vector.scalar_tensor_tensor(
                            y_acc[mo][:], pe_[:], bce[:, mo:mo + 1],
                            bc_sb[:], op0=OP.add, op1=OP.mult)
                    else:
                        yt = scratch.tile([128, TPC], F32, tag="yt")
                        nc.vector.scalar_tensor_tensor(
                            yt[:], pe_[:], bce[:, mo:mo + 1], bc_sb[:],
                            op0=OP.add, op1=OP.mult)
                        nc.gpsimd.tensor_tensor(y_acc[mo][:], y_acc[mo][:],
                                                yt[:], op=OP.add)
                    yield
            if DEBUG[0]:
                nc.sync.dma_start(dbg_y[:], y_acc[0][:])
            yield

        # adaptive interleave: pace expert chunks so both streams finish together
        import itertools
        conv_it = itertools.chain(rc_steps(), dbg_gates(), conv1_steps(),
                                  conv2_steps())
        N_STEPS = 6 + 1 + 4 + 1 + 4 * 19 + 17
        N_CHUNKS = E * 21 + 1
        ex_it = expert_chunks()
        steps_done = 0
        chunks_done = 0
        conv_live, ex_live = True, True
        while conv_live or ex_live:
            if conv_live:
                try:
                    next(conv_it)
                    steps_done += 1
                except StopIteration:
                    conv_live = False
            quota = (N_CHUNKS + 1) if not conv_live else int(
                N_CHUNKS * steps_done / (N_STEPS - 12))
            if steps_done < 12:
                # gatesT not fully emitted yet: emitting a gate-broadcast
                # chunk now would read a partially-written tile (the dep
                # tracker only sees writes emitted so far). Only gemm1
                # chunks (e0's first 16) are safe.
                quota = min(quota, 16)
            while ex_live and chunks_done < quota:
                try:
                    next(ex_it)
                    chunks_done += 1
                except StopIteration:
                    ex_live = False

        # ===== z = x + y + sh2 ; layernorm in z-layout (no transposes) =====
        z_sb = []
        for mo in range(4):
            z = acts.tile([128, TPC], F32R, tag=f"z{mo}", name=f"z_sb{mo}")
            nc.vector.tensor_tensor(z[:], y_acc[mo][:], sh2_sb[mo][:], op=OP.add)
            nc.vector.tensor_tensor(z[:], z[:],
                                    xt_sb[mo][:, C0:C0 + TPC], op=OP.add)
            z_sb.append(z)

        # column sums of z and z^2 over the 512 channels (4 partition tiles)
        psm = psum.tile([1, TPC], F32, tag="mm", bufs=4, name="psm")
        for mo in range(4):
            nc.tensor.matmul(psm[:], onesc_sb[:], z_sb[mo][:],
                             start=(mo == 0), stop=(mo == 3))
        psq = psum.tile([1, TPC], F32, tag="mm", bufs=4, name="psq")
        for mo in range(4):
            zsq = scratch.tile([128, TPC], F32R, tag="zsq", bufs=1, name="zsq")
            nc.scalar.activation(zsq[:], z_sb[mo][:], AF.Square)
            nc.tensor.matmul(psq[:], onesc_sb[:], zsq[:],
                             start=(mo == 0), stop=(mo == 3))
        mu_row = scratch.tile([1, TPC], F32R, tag="murow", bufs=1, name="mu_row")
        nc.vector.tensor_scalar(mu_row[:], psm[:], 1.0 / D_IN, None, op0=OP.mult)
        ez2 = scratch.tile([1, TPC], F32, tag="lnrow", name="ez2")
        nc.vector.tensor_scalar(ez2[:], psq[:], 1.0 / D_IN, 1e-5,
                                op0=OP.mult, op1=OP.add)
        mu2 = scratch.tile([1, TPC], F32, tag="lnrow", name="mu2")
        nc.vector.tensor_tensor(mu2[:], mu_row[:].bitcast(F32), mu_row[:].bitcast(F32),
                                op=OP.mult)
        vpe = scratch.tile([1, TPC], F32, tag="lnt1", bufs=1, name="vpe")
        nc.vector.scalar_tensor_tensor(vpe[:], mu2[:], -1.0, ez2[:],
                                       op0=OP.mult, op1=OP.add)
        rinv = scratch.tile([1, TPC], F32, tag="lnrow", name="rinv")
        nc.vector.reciprocal(rinv[:], vpe[:])
        rstd_row = scratch.tile([1, TPC], F32R, tag="rsrow", bufs=1, name="rstd_row")
        nc.scalar.activation(rstd_row[:], rinv[:], AF.Sqrt)
        # broadcast mu and rstd to 128 partitions
        pmu = psum.tile([128, TPC], F32, tag="mm", bufs=4, name="pmu")
        nc.tensor.matmul(pmu[:], ones_sb[:], mu_row[:], start=True, stop=True)
        mu_bc = scratch.tile([128, TPC], F32, tag="mubc", bufs=1, name="mu_bc")
        nc.scalar.copy(mu_bc[:], pmu[:])
        prs = psum.tile([128, TPC], F32, tag="mm", bufs=4, name="prs")
        nc.tensor.matmul(prs[:], ones_sb[:], rstd_row[:], start=True, stop=True)
        rs_bc = scratch.tile([128, TPC], F32, tag="rsbc", bufs=1, name="rs_bc")
        nc.scalar.copy(rs_bc[:], prs[:])
        for mo in range(4):
            t1 = scratch.tile([128, TPC], F32, tag="lnt1", bufs=1, name="t1")
            nc.vector.tensor_tensor(t1[:], z_sb[mo][:].bitcast(F32), mu_bc[:],
                                    op=OP.subtract)
            t2 = scratch.tile([128, TPC], F32, tag="lnt2", bufs=1, name="t2")
            nc.vector.tensor_tensor(t2[:], t1[:], rs_bc[:], op=OP.mult)
            out = scratch.tile([128, TPC], F32, tag="out", bufs=2, name="out")
            nc.vector.tensor_scalar(out[:], t2[:], lngc_sb[:, mo:mo + 1],
                                    lnbc2_sb[:, mo:mo + 1],
                                    op0=OP.mult, op1=OP.add)
            nc.sync.dma_start(yout[mo * 128:(mo + 1) * 128, :], out[:])

    _split_multi_waits(nc)
    return nc


_CACHE = {}


def _get_nc(reps=1):
    key = f"nc{reps}"
    if key not in _CACHE:
        _CACHE[key] = _build(reps)
    return _CACHE[key]


LAST_RESULT = {}


def kernel(x, rc_w1, rc_b1, rc_w2, rc_b2, sh_w1, sh_b1, sh_w2, sh_b2,
           e_w1, e_b1, e_w2, e_b2, ln_g, ln_b, **kwargs):
    import ml_dtypes
    FP8 = ml_dtypes.float8_e4m3
    FP8L = ml_dtypes.float8_e5m2
    BF = ml_dtypes.bfloat16

    x = np.asarray(x, np.float32)
    f = lambda a: np.ascontiguousarray(np.asarray(a, np.float32))

    def q8(a):
        return np.asarray(a, np.float32).astype(FP8)

    def qe5(a):
        return np.asarray(a, np.float32).astype(FP8L)

    # shared conv1: hi/lo packs [tap, kp, blk, p, j, mcol]
    w1 = np.asarray(sh_w1, np.float32)            # [2048, 512, 9]
    w1t = w1.transpose(2, 1, 0)                   # [9, 512, 2048]
    w1h = q8(w1t)
    w1l = qe5(w1t - w1h.astype(np.float32))
    def packw1(a):
        # [9, 512, 2048] -> [9, kp2, p128, j2, 2048]
        a = a.reshape(9, 2, 2, 128, 2048)         # [tap, kp, j, p, m]
        return np.ascontiguousarray(a.transpose(0, 1, 3, 2, 4))
    shw1h_a = packw1(w1h)
    shw1l_a = packw1(w1l)

    # shared conv2 bf16: [g, p, kk, d]
    w2 = np.asarray(sh_w2, np.float32)[:, :, 0]   # [512, 2048]
    shw2p_a = np.ascontiguousarray(
        w2.T.reshape(4, 4, 128, 512).transpose(0, 2, 1, 3)).astype(BF)

    # expert weights fp8 DoubleRow packs: [e, p, kp, j, m]
    ew1 = q8(e_w1)                                # [E, 512, 2048]
    ew1p_a = np.ascontiguousarray(
        ew1.reshape(E, 2, 2, 128, D_HID).transpose(0, 3, 1, 2, 4))
    ew2 = q8(e_w2)                                # [E, 2048, 512]
    ew2p_a = np.ascontiguousarray(
        ew2.reshape(E, 8, 2, 128, D_IN).transpose(0, 3, 1, 2, 4))

    shared = {
        "rcw1t": f(np.asarray(rc_w1, np.float32).transpose(2, 1, 0)),
        "rcw2t": f(np.asarray(rc_w2, np.float32).transpose(2, 1, 0)),
        "shw1h": shw1h_a,
        "shw1l": shw1l_a,
        "shw2p": shw2p_a,
        "ew1p": ew1p_a,
        "ew2p": ew2p_a,
        "rcb1": f(np.asarray(rc_b1, np.float32).reshape(4, 128).T),
        "rcb2": f(np.asarray(rc_b2, np.float32).reshape(E, 1)),
        "shb1": f(np.asarray(sh_b1, np.float32).reshape(16, 128).T),
        "shb2": f(np.asarray(sh_b2, np.float32).reshape(4, 128).T),
        "eb1p": f(np.asarray(e_b1, np.float32).reshape(E, 16, 128).transpose(0, 2, 1)),
        "ebcp": f(np.asarray(e_b2, np.float32).reshape(E, 4, 128).transpose(0, 2, 1)),
        "lng": f(np.asarray(ln_g, np.float32).reshape(1, D_IN)),
        "lnb": f(np.asarray(ln_b, np.float32).reshape(1, D_IN)),
        "ones": np.ones((1, 128), np.float32),
        "onesc": np.ones((128, 1), np.float32),
        "lngc": f(np.asarray(ln_g, np.float32).reshape(4, 128).T),
        "lnbc2": f(np.asarray(ln_b, np.float32).reshape(4, 128).T),
        "sel": np.repeat(np.eye(E, dtype=np.float32), 128, axis=1),
        "ident": np.eye(128, dtype=np.float32),
    }

    in_maps = []
    for c in range(NCORES):
        b, j = divmod(c, T // TPC)
        # f32r slab with HALO-wide halo
        lo_tok = j * TPC - HALO
        hi_tok = j * TPC + TPC + HALO
        xh = np.zeros((W, D_IN), np.float32)
        lo = max(0, lo_tok)
        hi = min(T, hi_tok)
        xh[lo - lo_tok: hi - lo_tok] = x[b, lo:hi]
        im = dict(shared)
        im["xt"] = np.ascontiguousarray(xh.T)
        # fp8 slab with H8-wide halo: [p, k, col]
        lo8, hi8 = j * TPC - H8, j * TPC + TPC + H8
        xh8 = np.zeros((W8, D_IN), np.float32)
        lo_c, hi_c = max(0, lo8), min(T, hi8)
        xh8[lo_c - lo8: hi_c - lo8] = x[b, lo_c:hi_c]
        xthi = q8(xh8.T)                          # [512, 520]
        xtlo = qe5(xh8.T - xthi.astype(np.float32))
        im["x8h"] = np.ascontiguousarray(xthi.reshape(4, 128, W8).transpose(1, 0, 2))
        im["x8l"] = np.ascontiguousarray(xtlo.reshape(4, 128, W8).transpose(1, 0, 2))
        rcm = np.zeros((1, 528), np.float32)
        for cidx in range(528):
            tok = j * TPC - 8 + cidx
            if 0 <= tok < T:
                rcm[0, cidx] = 1.0
        im["rcmask"] = rcm
        in_maps.append(im)

    nc = _get_nc()
    res = run_bass_kernel_spmd(nc, in_maps, core_ids=list(range(NCORES)),
                               **kwargs)
    LAST_RESULT["res"] = res

    out = np.empty((B, T, D_IN), np.float32)
    for c in range(NCORES):
        b, j = divmod(c, T // TPC)
        out[b, j * TPC:(j + 1) * TPC] = res.results[c]["yout"].T
    return out
